# revision 1
# baseline (speedup 1.0000x reference)
"""Trainium2 Bass kernel for nn_Decoder_1202590842919.

LSTM caption decoder: B=128 rows, T=32 decode steps, D=512, V=30000.

Key algorithmic facts used:
 - The length-sort + unsort in the reference is a per-row permutation that
   cancels: every batch row's output depends only on that row's data.
 - Masked (t >= len-1) steps produce zeros, and h/c freezing only affects
   steps that are themselves masked, so a dense (unfrozen) LSTM + zeroing
   of masked output rows is exact.

Structure (two SPMD launches on 8 cores):
 - Launch 1 "lstm": data-parallel over batch (16 rows/core). The x-side
   contribution (X @ W_ih.T + biases) is a per-octet GEMM whose matmuls are
   interleaved between the previous octet's recurrence steps (fills the PE
   while the serial gate/cell chain runs on ACT/DVE); the recurrence runs
   32 steps of [gates(2048) x batch(16)] matmuls with gates on partitions.
   All matmuls are bf16 (fp8 variants measured slower or no faster).
 - Launch 2 "fc": vocab-parallel (3750 cols/core), bf16 with bf16 output
   (host upcasts). Only the ~52% of (b, t) rows that are active are packed
   and projected; the host scatters them into a zero-filled full output.
   PSUM drains alternate between ACT and DVE; output DMAs ride the
   otherwise-idle sync/gpsimd queues.

Measured end-to-end relative error vs the fp32 reference: ~3.5e-3
(gate: 2e-2).
"""

import functools
import hashlib
import os
import shutil

import numpy as np
import ml_dtypes

import concourse.bass as bass
import concourse.bacc as bacc
import concourse.tile as tile
import concourse.mybir as mybir
from concourse.bass_utils import run_bass_kernel_spmd

BF16 = ml_dtypes.bfloat16
FP8 = ml_dtypes.float8_e4m3
FP32 = mybir.dt.float32
BF16_DT = mybir.dt.bfloat16
FP8_DT = mybir.dt.float8e4
WHH_SCALE = 1.0   # >1 only needed when W_hh is stored fp8
DR = False       # fp8 DoubleRow h-side: measured net loss at FD=16 (LDW +72%)

B = 128          # batch
TF = 33          # max caption len
T = TF - 1       # decode steps
V = 30000
E = 300
ENC = 256
D = 512
G4 = 4 * D       # gates
NC = 8           # cores
BL = B // NC     # batch rows per core
KX = E + ENC + 1           # x-side contraction (emb+enc+ones)
KXT = 3                    # x-side K tiles: emb only (pad 300 -> 384); the
                           # enc+bias part is constant over t and enters as a
                           # host-precomputed [16, G4] tile via an identity rhs
KH = 4                     # h-side K tiles (512)
MT = G4 // 128             # 16 gate M-tiles
VC = V // NC               # vocab cols per core (3750)
AF = mybir.ActivationFunctionType
OP = mybir.AluOpType


def _bf(x):
    return np.ascontiguousarray(x).astype(BF16)


def _install_neff_cache():
    """Disk-cache compiled NEFFs keyed on BIR bytes (BIR serialization is
    deterministic across processes) so repeated runs skip neuronx-cc."""
    import concourse.bass2jax as b2j
    if getattr(b2j, "_neff_cache_installed", False):
        return
    orig = b2j.compile_bir_kernel
    cache_dir = os.path.expanduser("~/.cache/bass_neff")
    os.makedirs(cache_dir, exist_ok=True)

    def cached(bir_json, tmpdir, neff_name="file.neff", **kw):
        h = hashlib.sha256(bir_json).hexdigest()[:32]
        cpath = os.path.join(cache_dir, h + ".neff")
        if os.path.exists(cpath):
            out = os.path.join(tmpdir, neff_name)
            shutil.copy(cpath, out)
            return out
        p = orig(bir_json, tmpdir, neff_name=neff_name, **kw)
        try:
            shutil.copy(p, cpath)
        except OSError:
            pass
        return p

    b2j.compile_bir_kernel = cached
    b2j._neff_cache_installed = True


_install_neff_cache()


@functools.cache
def build_lstm_nc(reps=1, dr=DR):
    """LSTM launch. dr=True runs the h-side matmuls in fp8 DoubleRow mode:
    one LDWEIGHTS+MATMUL pair covers K=256 (two K-tiles), halving the
    recurrence's instruction count (the measured bound is per-pair issue
    cost, not weight-load bandwidth). h is quantized to fp8 only for the
    recurrence input; fc still sees the bf16 h."""
    nc = bacc.Bacc("TRN2")
    xT = nc.declare_dram_parameter("xT", [128, KXT * T * BL], BF16_DT, isOutput=False)
    wih = nc.declare_dram_parameter("wih", [128, KXT * G4], BF16_DT, isOutput=False)
    # cgate[k, g] = (enc @ W_enc.T + b)[b=k, g]: per-batch-row constant gate
    # contribution; crh[k, (t,b)] = [b == k] selects it per output column
    cgate = nc.declare_dram_parameter("cgate", [BL, G4], BF16_DT, isOutput=False)
    crh = nc.declare_dram_parameter("crh", [BL, 8 * BL], BF16_DT, isOutput=False)
    whh = nc.declare_dram_parameter("whh", [128, KH * G4],
                                    FP8_DT if dr else BF16_DT, isOutput=False)
    h0 = nc.declare_dram_parameter("h0", [128, KH * BL], FP8_DT if dr else BF16_DT,
                                   isOutput=False)
    c0 = nc.declare_dram_parameter("c0", [128, KH * BL], FP32, isOutput=False)
    hout = nc.declare_dram_parameter("hout", [128, T * KH * BL], BF16_DT, isOutput=True)

    TB = T * BL  # 512 (t, b) columns
    W = KH * BL  # 64: free width of state tiles

    with tile.TileContext(nc) as tc:
        with tc.tile_pool(name="const", bufs=1) as cp:
            # DMA order = first-use order; weights stream on the gpsimd
            # queue, activations/state on the sync queue.
            xt_sb = cp.tile([128, KXT * TB], BF16_DT)
            nc.sync.dma_start(xt_sb[:], xT[:])
            cg_sb = cp.tile([BL, G4], BF16_DT)
            nc.sync.dma_start(cg_sb[:], cgate[:])
            crh_sb = cp.tile([BL, 8 * BL], BF16_DT)
            nc.sync.dma_start(crh_sb[:], crh[:])
            wih_sb = cp.tile([128, KXT * G4], BF16_DT)
            nc.gpsimd.dma_start(wih_sb[:, :G4], wih[:, :G4])
            whh_sb = cp.tile([128, KH * G4], FP8_DT if dr else BF16_DT)
            nc.gpsimd.dma_start(whh_sb[:], whh[:])
            h0_sb = cp.tile([128, W], FP8_DT if dr else BF16_DT)
            nc.sync.dma_start(h0_sb[:], h0[:])
            h8_sb = cp.tile([128, T * W], FP8_DT, name="h8_sb") if dr else None
            c_sb = cp.tile([128, W], FP32)
            nc.sync.dma_start(c_sb[:], c0[:])
            nc.gpsimd.dma_start(wih_sb[:, G4:], wih[:, G4:])
            hout_sb = cp.tile([128, T * W], BF16_DT)  # free = (t, dt, b)

            # Two PSUM "octet" tiles (4 banks each): the x-side GEMM for 8
            # steps lands in PSUM as [quad q][mt j][t][b]; the recurrence
            # then accumulates W_hh@h on top and ACT reads gates straight
            # from PSUM. Gate rows are host-permuted to [i, f, o, g] so one
            # sigmoid covers quads 0..2 and one tanh covers quad 3.
            with (
                tc.tile_pool(name="oct", bufs=2, space="PSUM") as octp,
                tc.tile_pool(name="vt", bufs=3) as vt,
            ):
              NOCT = T // 8

              def emit_x_mm(o, P, kt, mt):
                    q, j = divmod(mt, 4)
                    out = P[:, q * 512 + j * 128 : q * 512 + (j + 1) * 128]
                    if kt == KXT:
                        # constant enc+bias gate contribution: one K=16 tile
                        # (identity-select rhs) instead of 2 K-tiles of GEMM
                        nc.tensor.matmul(
                            out, lhsT=cg_sb[:, mt * 128 : (mt + 1) * 128],
                            rhs=crh_sb[:], start=False, stop=False,
                            skip_group_check=True,
                        )
                        return
                    nc.tensor.matmul(
                        out,
                        lhsT=wih_sb[:, kt * G4 + mt * 128 : kt * G4 + (mt + 1) * 128],
                        rhs=xt_sb[:, kt * TB + o * 128 : kt * TB + (o + 1) * 128],
                        # start=True zeroes the whole PSUM bank (zero-region
                        # granularity), so only the first matmul touching
                        # bank q may set it.
                        start=(j == 0 and kt == 0),
                        stop=False,
                        skip_group_check=True,
                    )

              P_cur = None
              for _rep in range(reps):
                hlast = h8_sb if dr else hout_sb
                hprev = h0_sb[:] if _rep == 0 else hlast[:, (T - 1) * W : T * W]
                whh_r = whh_sb[:].rearrange("p (k g) -> p k g", k=KH)
                # x-GEMM for octet o+1 is emitted in small blocks BETWEEN the
                # recurrence steps of octet o, so the PE streams x-matmuls
                # while the serial gate/cell chain of each step runs on
                # ACT/DVE. Octet 0's x-GEMM runs up front (kt-outer so the
                # first matmuls only need wih's kt0 slice).
                if P_cur is None:
                    P_cur = octp.tile([128, 2048], FP32, tag="oct", name="P0")
                    for kt in range(KXT + 1):
                        for mt in range(MT):
                            emit_x_mm(0, P_cur, kt, mt)
                for o in range(NOCT):
                    P = P_cur
                    Pr = P[:].rearrange("p (q j t b) -> p q j t b", q=4, j=4, t=8)
                    if o + 1 < NOCT or _rep + 1 < reps:
                        P_next = octp.tile([128, 2048], FP32, tag="oct", name="Pn")
                        on = (o + 1) % NOCT
                        xq = [(kt, mt) for kt in range(KXT + 1) for mt in range(MT)]
                    else:
                        P_next, xq = None, []
                    for tt in range(8):
                        t = o * 8 + tt
                        for mt in range(MT):
                            q, j = divmod(mt, 4)
                            base = q * 512 + j * 128 + tt * 16
                            if dr:
                                hprev_r = hprev.rearrange("p (k b) -> p k b", k=KH)
                                for pp in range(KH // 2):
                                    nc.tensor.matmul(
                                        P[:, base : base + BL],
                                        lhsT=whh_r[:, 2 * pp : 2 * pp + 2,
                                                   mt * 128 : (mt + 1) * 128],
                                        rhs=hprev_r[:, 2 * pp : 2 * pp + 2, :],
                                        start=False,
                                        stop=(pp == KH // 2 - 1),
                                        perf_mode=mybir.MatmulPerfMode.DoubleRow,
                                        skip_group_check=True,
                                    )
                            else:
                                for kt in range(KH):
                                    nc.tensor.matmul(
                                        P[:, base : base + BL],
                                        lhsT=whh_sb[:, kt * G4 + mt * 128 : kt * G4 + (mt + 1) * 128],
                                        rhs=hprev[:, kt * BL : (kt + 1) * BL],
                                        start=False,
                                        stop=(kt == KH - 1),
                                        skip_group_check=True,
                                    )

                        # next octet's x-matmuls fill the PE while this
                        # step's gate/cell chain runs on ACT/DVE
                        for _ in range(10):
                            if xq:
                                kt, mt = xq.pop(0)
                                emit_x_mm(on, P_next, kt, mt)

                        # gate quads are host-permuted to [i, f, o, g]
                        sfio = vt.tile([128, 3 * W], FP32, tag="sfio")
                        nc.scalar.activation(
                            sfio[:].rearrange("p (q j b) -> p q j b", q=3, j=4),
                            Pr[:, 0:3, :, tt, :],
                            AF.Sigmoid,
                            scale=1.0 / WHH_SCALE,
                        )
                        tg = vt.tile([128, W], FP32, tag="tg")
                        nc.scalar.activation(
                            tg[:].rearrange("p (j b) -> p j b", j=4),
                            Pr[:, 3, :, tt, :],
                            AF.Tanh,
                            scale=1.0 / WHH_SCALE,
                        )
                        si = sfio[:, 0:W]
                        sf = sfio[:, W : 2 * W]
                        so = sfio[:, 2 * W : 3 * W]
                        m1 = vt.tile([128, W], FP32, tag="m1")
                        nc.vector.tensor_tensor(m1[:], si, tg[:], OP.mult)
                        m2 = vt.tile([128, W], FP32, tag="m2")
                        nc.vector.tensor_tensor(m2[:], sf, c_sb[:], OP.mult)
                        nc.vector.tensor_tensor(c_sb[:], m1[:], m2[:], OP.add)
                        tcn = vt.tile([128, W], FP32, tag="tc")
                        nc.scalar.activation(tcn[:], c_sb[:], AF.Tanh)
                        hslot = hout_sb[:, t * W : (t + 1) * W]
                        if dr:
                            # fp8 h feeds the next step's DoubleRow matmul
                            # (critical path); the precise bf16 copy for fc
                            # follows off-path.
                            h8slot = h8_sb[:, t * W : (t + 1) * W]
                            nc.vector.tensor_tensor(h8slot, so, tcn[:], OP.mult)
                            nc.vector.tensor_tensor(hslot, so, tcn[:], OP.mult)
                            hprev = h8slot
                        else:
                            nc.vector.tensor_tensor(hslot, so, tcn[:], OP.mult)
                            hprev = hslot

                    # stream this octet's h out as soon as it's done (gpsimd
                    # queue: idle here, so the wait can't block compute)
                    if _rep == reps - 1:
                        nc.gpsimd.dma_start(hout[:, o * 8 * W : (o + 1) * 8 * W],
                                            hout_sb[:, o * 8 * W : (o + 1) * 8 * W])
                    P_cur = P_next
    nc.finalize()
    return nc


@functools.cache
def build_fc_nc(npad, zero_bias=True, reps=1, nsweep=1):
    nc = bacc.Bacc("TRN2")
    hact = nc.declare_dram_parameter("hact", [128, KH * npad], BF16_DT, isOutput=False)
    wfc = nc.declare_dram_parameter("wfc", [128, KH * VC], BF16_DT, isOutput=False)
    if not zero_bias:
        bias = nc.declare_dram_parameter("bias", [128, VC], BF16_DT, isOutput=False)
    pred = nc.declare_dram_parameter("pred", [npad, VC], BF16_DT, isOutput=True)

    chunks = []
    v0 = 0
    while v0 < VC:
        w = min(512, VC - v0)
        chunks.append((v0, w))
        v0 += w

    with tile.TileContext(nc) as tc:
        with (
            tc.tile_pool(name="const", bufs=1) as cp,
            tc.tile_pool(name="ps", bufs=4, space="PSUM") as pp,
            tc.tile_pool(name="ob", bufs=4) as op_,
        ):
            # DMA order = first-use order, few big strided DMAs: mt0's hact
            # (all kt, one 3D-AP DMA), wfc chunk0 (all kt, one DMA), the hact
            # remainder, then the remaining wfc chunks chunk-major. hact on
            # the sync queue, wfc on the gpsimd queue so they stream in
            # parallel.
            hact_sb = cp.tile([128, KH * npad], BF16_DT)
            hr = lambda ap: ap.rearrange("p (k n) -> p k n", k=KH)
            wr = lambda ap: ap.rearrange("p (k v) -> p k v", k=KH)
            nc.sync.dma_start(hr(hact_sb[:])[:, :, :128], hr(hact[:])[:, :, :128])
            if not zero_bias:
                bias_sb = cp.tile([128, VC], BF16_DT)
                nc.sync.dma_start(bias_sb[:], bias[:])
            wfc_sb = cp.tile([128, KH * VC], BF16_DT)
            for v0, w in chunks[:nsweep]:
                nc.gpsimd.dma_start(wr(wfc_sb[:])[:, :, v0 : v0 + w],
                                    wr(wfc[:])[:, :, v0 : v0 + w])
            hmid = min(1152, npad)
            nc.sync.dma_start(hr(hact_sb[:])[:, :, 128:hmid], hr(hact[:])[:, :, 128:hmid])
            if hmid < npad:
                nc.sync.dma_start(hr(hact_sb[:])[:, :, hmid:], hr(hact[:])[:, :, hmid:])
            for v0, w in chunks[nsweep:]:
                nc.gpsimd.dma_start(wr(wfc_sb[:])[:, :, v0 : v0 + w],
                                    wr(wfc[:])[:, :, v0 : v0 + w])

            NMT = npad // 128

            def mm_chunk(mt, ci, ob, obase):
                v0, w = chunks[ci]
                ps = pp.tile([128, 512], FP32)
                for kt in range(KH):
                    nc.tensor.matmul(
                        ps[:, :w],
                        lhsT=hact_sb[:, kt * npad + mt * 128 : kt * npad + (mt + 1) * 128],
                        rhs=wfc_sb[:, kt * VC + v0 : kt * VC + v0 + w],
                        start=(kt == 0),
                        stop=(kt == KH - 1),
                    )
                # PSUM -> SBUF drain (fp32 -> bf16), alternating ACT/DVE
                # so neither engine becomes the bottleneck
                o = ob[:, v0 - obase : v0 - obase + w]
                if zero_bias:
                    if (mt + ci) % 2 == 0:
                        nc.scalar.activation(o, ps[:, :w], AF.Copy)
                    else:
                        nc.vector.tensor_copy(o, ps[:, :w])
                else:
                    nc.vector.tensor_tensor(o, ps[:, :w], bias_sb[:, v0 : v0 + w], OP.add)

            for _rep in range(reps):
                # leading-chunk sweeps over all M-tiles: matmuls against wfc
                # chunks that are resident early, hiding the wfc/hact stream.
                # Output DMAs go on the scalar/vector queues (sync + gpsimd
                # carry the input streams).
                w0 = sum(w for _, w in chunks[:nsweep])
                for ci in range(nsweep):
                    for mt in range(NMT):
                        c0 = op_.tile([128, 512], BF16_DT, tag="c0")
                        v0, w = chunks[ci]
                        mm_chunk(mt, ci, c0, v0)
                        nc.sync.dma_start(
                            pred[mt * 128 : (mt + 1) * 128, v0 : v0 + w], c0[:, :w])
                # remaining chunks, M-tile-major. Row DMAs go on the gpsimd
                # queue: it is idle during the compute loop, so the DMA's
                # input-ready wait never head-of-line-blocks a compute engine.
                for mt in range(NMT):
                    ob = op_.tile([128, VC - w0], BF16_DT, tag="ob")
                    for ci in range(nsweep, len(chunks)):
                        mm_chunk(mt, ci, ob, w0)
                    nc.gpsimd.dma_start(pred[mt * 128 : (mt + 1) * 128, w0:], ob[:])
    nc.finalize()
    return nc


def prep_launch1(ref_obj_features, lang_indices, idx2embedding,
                 W_ih, b_ih, W_hh, b_hh, W_init_h, b_init_h, W_init_c, b_init_c):
    f32 = np.float32
    enc = np.asarray(ref_obj_features, f32)
    lang_indices = np.asarray(lang_indices)

    emb = np.asarray(idx2embedding, f32)[lang_indices[:, :T]]      # [B, T, E]
    h0 = enc @ np.asarray(W_init_h, f32).T + np.asarray(b_init_h, f32)
    c0 = enc @ np.asarray(W_init_c, f32).T + np.asarray(b_init_c, f32)

    # X[t, b, :] = emb_t only, zero-padded to 384 rows of X.T; the enc+bias
    # part is constant over t and handled by the per-row cgate tile
    Xp = np.zeros((T, B, KXT * 128), f32)
    Xp[:, :, :E] = emb.transpose(1, 0, 2)

    # permute gate rows to [i, f, o, g]: one sigmoid covers quads 0..2.
    # All gate contributions are pre-scaled by WHH_SCALE (undone by the ACT
    # `scale` on the gate nonlinearities) so W_hh lands in fp8's normal range.
    perm = np.r_[0:D, D : 2 * D, 3 * D : 4 * D, 2 * D : 3 * D]
    W_ih = np.asarray(W_ih, f32)
    Wx = np.zeros((KXT * 128, G4), f32)
    Wx[:E] = W_ih.T[:E][:, perm] * WHH_SCALE
    wih_arr = _bf(Wx.reshape(KXT, 128, G4).transpose(1, 0, 2).reshape(128, KXT * G4))
    # C[b, g] = enc_b @ W_enc.T + b_ih + b_hh, in permuted gate order
    Cg = (enc @ W_ih[:, E:].T + np.asarray(b_ih, f32)
          + np.asarray(b_hh, f32))[:, perm] * WHH_SCALE          # [B, G4]
    # crh[k, t*BL + b] = [b == k]
    crh_arr = _bf(np.tile(np.eye(BL, dtype=f32), (1, 8)))
    whh_arr = np.ascontiguousarray(
        (np.asarray(W_hh, f32).T[:, perm] * WHH_SCALE).reshape(KH, 128, G4)
        .transpose(1, 0, 2).reshape(128, KH * G4)).astype(FP8 if DR else BF16)

    in_maps1 = []
    for c in range(NC):
        sl = slice(c * BL, (c + 1) * BL)
        xT_c = _bf(Xp[:, sl, :].reshape(T, BL, KXT, 128)
                   .transpose(3, 2, 0, 1).reshape(128, KXT * T * BL))
        h0_c = np.ascontiguousarray(
            h0[sl].reshape(BL, KH, 128).transpose(2, 1, 0).reshape(128, KH * BL)
        ).astype(FP8 if DR else BF16)
        c0_c = np.ascontiguousarray(
            c0[sl].reshape(BL, KH, 128).transpose(2, 1, 0).reshape(128, KH * BL), f32)
        in_maps1.append({"xT": xT_c, "wih": wih_arr, "whh": whh_arr,
                         "cgate": _bf(Cg[sl]), "crh": crh_arr,
                         "h0": h0_c, "c0": c0_c})
    return in_maps1


def assemble_H(res1):
    """[KH, 128, B*T] bf16 (d-tiled, row = b*T + t)."""
    H_dtb = np.empty((KH, 128, B, T), BF16)
    for c in range(NC):
        hc = res1[c]["hout"].reshape(128, T, KH, BL)          # (p, t, dt, b)
        H_dtb[:, :, c * BL : (c + 1) * BL, :] = hc.transpose(2, 0, 3, 1)
    return H_dtb.reshape(KH, 128, B * T)


def active_rows(lang_len):
    dec_len = np.asarray(lang_len).astype(np.int64) - 1
    mask = np.arange(T)[None, :] < dec_len[:, None]            # [B, T]
    active = np.flatnonzero(mask.reshape(-1))                  # b-major, t within
    npad = max(128, ((len(active) + 127) // 128) * 128)
    return active, npad


def prep_launch2(H_flat, active, npad, W_fc, b_fc):
    f32 = np.float32
    hact_all = np.zeros((KH, 128, npad), BF16)
    hact_all[:, :, : len(active)] = H_flat[:, :, active]
    hact_arr = np.ascontiguousarray(
        hact_all.transpose(1, 0, 2).reshape(128, KH * npad))

    W_fc = np.asarray(W_fc, f32)
    b_fc = np.asarray(b_fc, f32)
    zero_bias = not np.any(b_fc)
    in_maps2 = []
    for c in range(NC):
        vsl = slice(c * VC, (c + 1) * VC)
        wfc_c = _bf(W_fc[vsl].T.reshape(KH, 128, VC).transpose(1, 0, 2)
                    .reshape(128, KH * VC))
        m = {"hact": hact_arr, "wfc": wfc_c}
        if not zero_bias:
            m["bias"] = _bf(np.broadcast_to(b_fc[vsl], (128, VC)))
        in_maps2.append(m)
    return in_maps2


def scatter_out(res2, active):
    out = np.zeros((B * T, V), np.float32)
    n_act = len(active)
    for c in range(NC):
        out[active, c * VC : (c + 1) * VC] = res2[c]["pred"][:n_act].astype(np.float32)
    return out.reshape(B, T, V)


def kernel(ref_obj_features, lang_indices, lang_len, idx2embedding,
           W_ih, b_ih, W_hh, b_hh, W_init_h, b_init_h, W_init_c, b_init_c,
           W_fc, b_fc):
    in_maps1 = prep_launch1(ref_obj_features, lang_indices, idx2embedding,
                            W_ih, b_ih, W_hh, b_hh,
                            W_init_h, b_init_h, W_init_c, b_init_c)
    nc1 = build_lstm_nc(dr=DR)
    res1 = run_bass_kernel_spmd(nc1, in_maps1, list(range(NC))).results

    H_flat = assemble_H(res1)
    active, npad = active_rows(lang_len)
    in_maps2 = prep_launch2(H_flat, active, npad, W_fc, b_fc)

    nc2 = build_fc_nc(npad, zero_bias="bias" not in in_maps2[0])
    res2 = run_bass_kernel_spmd(nc2, in_maps2, list(range(NC))).results
    return scatter_out(res2, active)

